# revision 1
# baseline (speedup 1.0000x reference)
"""Trainium2 Bass kernel for nn_DeformableBlock (deformable conv v1 block).

Contract: kernel(**inputs) takes FULL unsharded inputs, returns FULL output.
Sharding: data-parallel over batch (B=8 -> 8 NeuronCores, 1 batch each),
weights replicated.

Per-core algorithm (one batch, Cin=128, Cout=256, H=W=64):
  1. offset conv (3x3, pad 1) as 9 shifted fp32r matmuls -> offset [18, 4096]
  2. PE-transpose offsets to pixel-major [128 jp, 32 jt, 18]; compute bilinear
     gather indices + tap weights with DVE ops (floor via int-cast trick;
     x-taps gathered as adjacent row-pairs with clamp/swap weight logic)
  3. indices through DRAM into the dma_gather "wrapped" layout via HWDGE
     xbar transposes (the wrap is a 128x16 int16 transpose) + group replicate
  4. build xT [4096, 128] bf16 in DRAM (PE transposes), then per (k, dy):
     dma_gather(transpose=False, elem=256 bf16 = x-pair) -> pixel-major
     G [128 jp, 16 jtl, 2*128] bf16 (1 descriptor per gathered pair)
  5. tap weighting with per-partition scalars (tensor_scalar +
     scalar_tensor_tensor accumulate) in pixel-major; PE transposes the
     [jp, c] tile to channel-major PSUM; ACT evacuates into val_k
  6. main conv: out[o, j] = sum_k W_k[o,:] @ val_k  (bf16 matmuls, fp32 PSUM)
  7. bias + ReLU on ScalarE, DMA out [256, 4096] f32
"""
import os
import sys
import numpy as np

try:
    import concourse.bass as bass
except ImportError:  # pragma: no cover
    sys.path.insert(0, '/opt/trn_rl_repo')
    import concourse.bass as bass
import concourse.bacc as bacc

import concourse.mybir as mybir
import concourse.tile as tile
from concourse import library_config
from concourse.bass_utils import run_bass_kernel_spmd

F32 = mybir.dt.float32
F32R = mybir.dt.float32r
BF16 = mybir.dt.bfloat16
I32 = mybir.dt.int32
I16 = mybir.dt.int16
ALU = mybir.AluOpType
ACTF = mybir.ActivationFunctionType

B, CIN, COUT, H, W = 8, 128, 256, 64, 64
HW = H * W          # 4096
NJT = HW // 128     # 32 pixel-major tiles
NK = 9
KY = [(-1), (-1), (-1), 0, 0, 0, 1, 1, 1]
KX = [(-1), 0, 1, (-1), 0, 1, (-1), 0, 1]
NHALF = 2
JH = HW // NHALF    # 2048 pixels per half

_CACHE = {}


def _split_multiwaits(nc, max_waits=1, kinds=None):
    """walrus CoreV3 codegen rejects control instructions carrying more
    than one sem-wait; split the excess into a chain of same-engine
    drains placed directly before the offender."""
    if kinds is None:
        kinds = (mybir.InstDrain,)
    n_split = 0
    for fn in nc.m.functions:
        for bb in fn.blocks:
            insts = list(bb.instructions)
            new = []
            changed = False
            for inst in insts:
                si = inst.sync_info
                if (isinstance(inst, kinds) and si is not None
                        and len(si.on_wait) > max_waits):
                    waits = list(si.on_wait)
                    pre, rest = waits[:-max_waits], waits[-max_waits:]
                    for i in range(0, len(pre), max_waits):
                        chunk = pre[i:i + max_waits]
                        d = mybir.InstDrain(
                            name=f"{inst.name}-wsplit{i}",
                            engine=inst.engine,
                            ins=[], outs=[],
                            sync_info=mybir.SyncInfo(
                                on_wait=chunk, on_update=[]),
                        )
                        new.append(d)
                        n_split += 1
                    inst.sync_info = mybir.SyncInfo(
                        on_wait=rest, on_update=list(si.on_update))
                    changed = True
                new.append(inst)
            if changed:
                bb.instructions = new
    return n_split


def _build_program(phase=3):
    nc = bacc.Bacc('TRN2', target_bir_lowering=False, debug=False,
                   enable_asserts=False, num_devices=B)

    # ---- DRAM I/O ----
    xp_d = nc.dram_tensor('xp', [CIN, 66 * 66], F32, kind='ExternalInput')
    woffT_d = nc.dram_tensor('woffT', [9, CIN, 18], BF16, kind='ExternalInput')
    boff_d = nc.dram_tensor('boff', [18, 1], F32, kind='ExternalInput')
    wdefT_d = nc.dram_tensor('wdefT', [NK, CIN, COUT], BF16, kind='ExternalInput')
    bdef_d = nc.dram_tensor('bdef', [128, 2], F32, kind='ExternalInput')
    ident_d = nc.dram_tensor('ident', [128, 128], F32, kind='ExternalInput')
    hgk_d = nc.dram_tensor('hgk', [128, NJT, NK], F32, kind='ExternalInput')
    wgk_d = nc.dram_tensor('wgk', [128, NJT, NK], F32, kind='ExternalInput')
    y_d = nc.dram_tensor('y', [COUT, HW], F32, kind='ExternalOutput')

    # DRAM scratch
    xT_d = nc.dram_tensor('xT_scratch', [HW + 2, CIN], BF16, kind='Internal')
    idxnat_d = nc.dram_tensor('idxnat', [NK, 2, HW], I16, kind='Internal')

    with tile.TileContext(nc) as tc:
        with (
            tc.tile_pool(name='const', bufs=1) as cpool,
            tc.tile_pool(name='ps_small', bufs=2, space='PSUM') as ps_small,
            tc.tile_pool(name='ps_conv', bufs=4, space='PSUM') as ps_conv,
        ):
            # persistent small tensors
            wdefT = cpool.tile([CIN, NK, COUT], BF16, tag='wdefT')
            nc.sync.dma_start(
                wdefT[:], wdefT_d.ap().rearrange('k c o -> c k o'))
            bdef = cpool.tile([128, 2], F32, tag='bdef')
            nc.sync.dma_start(bdef[:], bdef_d.ap())
            idxwr = cpool.tile([128, NK, 2, NHALF, JH // 16], I16, tag='idxwr')
            w4 = cpool.tile([128, NJT, NK, 4], F32, tag='w4')
            identb = cpool.tile([128, 128], BF16, tag='identb')

            # ======== phase 1: offsets, indices, weights, xT ========
            with tc.tile_pool(name='p1', bufs=1) as apool:
                x_sb = apool.tile([CIN, 66 * 66], F32, tag='x_sb')
                nc.sync.dma_start(x_sb[:], xp_d.ap())
                xbf = apool.tile([CIN, 66 * 66], BF16, tag='xbf')
                nc.vector.tensor_copy(xbf[:], x_sb[:])
                ident = apool.tile([128, 128], F32, tag='ident')
                nc.sync.dma_start(ident[:], ident_d.ap())
                woffT = apool.tile([CIN, 9, 18], BF16, tag='woffT')
                nc.sync.dma_start(
                    woffT[:], woffT_d.ap().rearrange('s c o -> c s o'))
                boff = apool.tile([18, 1], F32, tag='boff')
                nc.sync.dma_start(boff[:], boff_d.ap())
                hgk = apool.tile([128, NJT, NK], F32, tag='hgk')
                nc.sync.dma_start(hgk[:], hgk_d.ap())
                wgk = apool.tile([128, NJT, NK], F32, tag='wgk')
                nc.sync.dma_start(wgk[:], wgk_d.ap())
                nc.vector.tensor_copy(identb[:], ident[:])

                # ---------- offset conv: off [18, 4096] f32 ----------
                # Conv runs on the padded 66-wide grid so the streaming
                # operand is a single contiguous run; the interior is
                # extracted in the ACT epilogue (multi-dim APs are fine
                # on ACT, just not on the PE streaming side).
                off_sb = apool.tile([18, HW], F32, tag='off_sb')
                chunks = [(1 + 7 * i, 7) for i in range(9)] + [(64, 1)]
                for r0, nr in chunks:
                    nfree = 66 * (nr - 1) + 64
                    ps = ps_small.tile([18, 512], F32, tag='ps', name='ps')
                    for s in range(9):
                        dh, dw = s // 3, s % 3
                        beg = r0 * 66 + 1 + (dh - 1) * 66 + (dw - 1)
                        rhs = bass.AP(
                            tensor=xbf[:].tensor,
                            offset=xbf[:].offset + beg,
                            ap=[list(xbf[:].ap[0]), [1, nfree]],
                        )
                        nc.tensor.matmul(
                            ps[:, :nfree], lhsT=woffT[:, s, :], rhs=rhs,
                            start=(s == 0), stop=(s == 8))
                    src_in = bass.AP(
                        tensor=ps[:].tensor, offset=ps[:].offset,
                        ap=[list(ps[:].ap[0]), [66, nr], [1, 64]])
                    nc.scalar.activation(
                        off_sb[:, 64 * (r0 - 1):64 * (r0 - 1 + nr)], src_in,
                        ACTF.Identity, bias=boff[:], scale=1.0)

                # ---------- transpose offsets to pixel-major ----------
                offT = apool.tile([128, NJT, 18], F32, tag='offT')
                for jt in range(NJT):
                    ps = ps_small.tile([128, 18], F32, tag='ps')
                    nc.tensor.transpose(
                        ps[:], off_sb[:, 128 * jt:128 * (jt + 1)],
                        ident[:18, :18])
                    nc.vector.tensor_copy(offT[:, jt, :], ps[:])

                # ---------- xT build (bf16, [4096, 128] in DRAM) ----------
                xc = apool.tile([CIN, HW], BF16, tag='xc')
                xin_all = bass.AP(
                    tensor=xbf[:].tensor,
                    offset=xbf[:].offset + 67,
                    ap=[list(xbf[:].ap[0]), [66, 64], [1, 64]])
                nc.vector.tensor_copy(xc[:], xin_all)
                xTsb = apool.tile([128, NJT, CIN], BF16, tag='xTsb')
                for jt in range(NJT):
                    ps = ps_small.tile([128, 128], BF16, tag='psb', name='ps')
                    nc.tensor.transpose(
                        ps[:], xc[:, 128 * jt:128 * (jt + 1)], identb[:])
                    nc.vector.tensor_copy(xTsb[:, jt, :], ps[:])
                nc.sync.dma_start(
                    xT_d.ap()[:HW, :].rearrange('(jt jp) c -> jp jt c', jp=128),
                    xTsb[:])
                zpad = apool.tile([1, 2 * CIN], BF16, tag='zpad')
                nc.vector.memset(zpad[:], 0)
                nc.sync.dma_start(
                    xT_d.ap()[HW:HW + 2, :].rearrange('a c -> (a c)').unsqueeze(0),
                    zpad[:])

                # ---------- index/weight arithmetic (pixel-major) ----------
                sh = [128, NJT, NK]

                def T(tag, dt=F32):
                    return apool.tile(sh, dt, tag=tag, name=tag)

                dyx = offT[:].rearrange('p jt (k two) -> p jt k two', two=2)
                dy = dyx[:, :, :, 0]
                dx = dyx[:, :, :, 1]

                ti = apool.tile(sh, I32, tag='ti')
                fdy, fdx = T('fdy'), T('fdx')
                tmp1, tmp2, tmp3 = T('tmp1'), T('tmp2'), T('tmp3')
                # floor(dy)
                nc.vector.tensor_copy(ti[:], dy)
                nc.vector.tensor_copy(fdy[:], ti[:])
                nc.vector.tensor_tensor(tmp1[:], fdy[:], dy, ALU.is_gt)
                nc.vector.tensor_tensor(fdy[:], fdy[:], tmp1[:], ALU.subtract)
                # floor(dx)
                nc.vector.tensor_copy(ti[:], dx)
                nc.vector.tensor_copy(fdx[:], ti[:])
                nc.vector.tensor_tensor(tmp1[:], fdx[:], dx, ALU.is_gt)
                nc.vector.tensor_tensor(fdx[:], fdx[:], tmp1[:], ALU.subtract)

                ly, lx = T('ly'), T('lx')
                nc.vector.tensor_tensor(ly[:], dy, fdy[:], ALU.subtract)
                nc.vector.tensor_tensor(lx[:], dx, fdx[:], ALU.subtract)

                y0, x0 = T('y0'), T('x0')
                nc.vector.tensor_tensor(y0[:], hgk[:], fdy[:], ALU.add)
                nc.vector.tensor_tensor(x0[:], wgk[:], fdx[:], ALU.add)

                yc0, yc1, y01 = T('yc0'), T('yc1'), T('y01')
                nc.vector.tensor_scalar(yc0[:], y0[:], 63.0, 0.0, ALU.min, ALU.max)
                nc.vector.tensor_scalar(y01[:], y0[:], 1.0, None, ALU.add)
                nc.vector.tensor_scalar(yc1[:], y01[:], 63.0, 0.0, ALU.min, ALU.max)
                vy0, vy1 = T('vy0'), T('vy1')
                nc.vector.tensor_tensor(vy0[:], y0[:], yc0[:], ALU.is_equal)
                nc.vector.tensor_tensor(vy1[:], y01[:], yc1[:], ALU.is_equal)

                bx, x01, e0, e1, e3 = T('bx'), T('x01'), T('e0'), T('e1'), T('e3')
                nc.vector.tensor_scalar(bx[:], x0[:], 62.0, 0.0, ALU.min, ALU.max)
                nc.vector.tensor_scalar(x01[:], x0[:], 1.0, None, ALU.add)
                nc.vector.tensor_tensor(e0[:], x0[:], bx[:], ALU.is_equal)
                nc.vector.tensor_tensor(e1[:], x01[:], bx[:], ALU.is_equal)
                nc.vector.tensor_scalar(tmp1[:], bx[:], 1.0, None, ALU.add)
                nc.vector.tensor_tensor(e3[:], x0[:], tmp1[:], ALU.is_equal)

                wy0, wy1 = T('wy0'), T('wy1')
                nc.vector.tensor_tensor(tmp1[:], ly[:], vy0[:], ALU.mult)
                nc.vector.tensor_tensor(wy0[:], vy0[:], tmp1[:], ALU.subtract)
                nc.vector.tensor_tensor(wy1[:], ly[:], vy1[:], ALU.mult)

                wx0, wx1 = T('wx0'), T('wx1')
                nc.vector.tensor_tensor(tmp1[:], lx[:], e0[:], ALU.mult)
                nc.vector.tensor_tensor(tmp2[:], lx[:], e1[:], ALU.mult)
                nc.vector.tensor_tensor(wx0[:], e0[:], tmp1[:], ALU.subtract)
                nc.vector.tensor_tensor(wx0[:], wx0[:], tmp2[:], ALU.add)
                nc.vector.tensor_tensor(tmp2[:], lx[:], e3[:], ALU.mult)
                nc.vector.tensor_tensor(tmp3[:], e3[:], tmp2[:], ALU.subtract)
                nc.vector.tensor_tensor(wx1[:], tmp1[:], tmp3[:], ALU.add)

                # tap weights -> w4 [128, NJT, NK, 4] f32 (persistent)
                nc.vector.tensor_tensor(w4[:, :, :, 0], wy0[:], wx0[:], ALU.mult)
                nc.vector.tensor_tensor(w4[:, :, :, 1], wy0[:], wx1[:], ALU.mult)
                nc.vector.tensor_tensor(w4[:, :, :, 2], wy1[:], wx0[:], ALU.mult)
                nc.vector.tensor_tensor(w4[:, :, :, 3], wy1[:], wx1[:], ALU.mult)

                # gather indices -> id0/id1 [128, NJT, NK] f32 (exact ints)
                id0, id1 = T('id0'), T('id1')
                nc.vector.tensor_scalar(tmp1[:], yc0[:], 64.0, None, ALU.mult)
                nc.vector.tensor_tensor(id0[:], tmp1[:], bx[:], ALU.add)
                nc.vector.tensor_scalar(tmp1[:], yc1[:], 64.0, None, ALU.mult)
                nc.vector.tensor_tensor(id1[:], tmp1[:], bx[:], ALU.add)

                # ---------- un-transpose idx to j-natural via PE ----------
                idx_tr = apool.tile([NJT, NK * 2, 128], I16, tag='idx_tr')
                for k in range(NK):
                    for d, idsrc in ((0, id0), (1, id1)):
                        ps = ps_small.tile([NJT, 128], F32, tag='ps')
                        nc.tensor.transpose(ps[:], idsrc[:, :, k], ident[:])
                        nc.vector.tensor_copy(idx_tr[:, 2 * k + d, :], ps[:])

                # ---------- reformat via DRAM ----------
                nc.sync.dma_start(
                    idxnat_d.ap().rearrange('k d (jt jp) -> jt (k d) jp', jp=128),
                    idx_tr[:])

                # wrapped idx: [128, NK, 2dy, NHALF, JH/16] int16.
                # The wrap (partition p = i%16, free f = i//16) is a 128x16
                # transpose of the j-natural rows -> HWDGE xbar transpose,
                # then replicate partitions 0-15 to the other 7 groups.
                for k in range(NK):
                    for d in range(2):
                        for half in range(NHALF):
                            src = idxnat_d.ap()[k, d,
                                                half * JH:(half + 1) * JH]
                            nc.sync.dma_start_transpose(
                                idxwr[0:16, k, d, half, :],
                                src.rearrange('(f p) -> f p', p=16))
                for g in range(1, 8):
                    nc.sync.dma_start(idxwr[16 * g:16 * (g + 1)], idxwr[0:16])

            if phase != 1:
                _phase2(nc, tc, ps_small, ps_conv, wdefT, bdef, idxwr, w4,
                        identb, xT_d, y_d, phase)
            if phase != 3:
                with tc.tile_pool(name='zz', bufs=1) as zp:
                    zt = zp.tile([128, HW], F32, tag='zt')
                    nc.vector.memset(zt[:], 0)
                    for oh in range(2):
                        nc.sync.dma_start(
                            y_d.ap()[128 * oh:128 * (oh + 1), :], zt[:])

    nc.finalize()
    _split_multiwaits(nc)
    return nc


def _phase2(nc, tc, ps_small, ps_conv, wdefT, bdef, idxwr, w4, identb,
            xT_d, y_d, phase=3):
            # ======== phase 2: gather + weighting + conv ========
            # Pixel-major pair-gather (1 descriptor per token), per-partition
            # scalar tap weighting (scalar_tensor_tensor), PE transpose to
            # channel-major, ACT evacuates PSUM into val.
            with (
                tc.tile_pool(name='gath', bufs=2) as gpool,
                tc.tile_pool(name='tmp2p', bufs=3) as tpool,
                tc.tile_pool(name='val', bufs=10) as vpool,
                tc.tile_pool(name='outp', bufs=2) as opool,
            ):
                for half in range(NHALF):
                    j0 = half * JH
                    vals = []
                    for k in range(NK):
                        g2 = []
                        for dyi in range(2):
                            # G [128 jp, 16 jtl, 2*CIN] bf16; token i ->
                            # partition i%128 = jp, chunk i//128 = local jt
                            G = gpool.tile([128, JH // 128, 2 * CIN], BF16,
                                           tag=f'G{dyi}', name=f'G{dyi}')
                            in_ap = bass.AP(
                                tensor=xT_d, offset=0,
                                ap=[[CIN, HW], [1, 2 * CIN]],
                            )
                            nc.gpsimd.dma_gather(
                                out_ap=G[:],
                                in_ap=in_ap,
                                idxs_ap=idxwr[:, k, dyi, half, :],
                                num_idxs=JH,
                                num_idxs_reg=JH,
                                elem_size=2 * CIN,
                                elem_step=CIN,
                                transpose=False,
                                single_packet=False,
                            )
                            g2.append(G)
                        if phase == 21:
                            continue
                        val = vpool.tile([128, JH], BF16, tag='val')
                        for jtl in range(JH // 128):
                            jt = half * (JH // 128) + jtl
                            acc = tpool.tile([128, CIN], BF16, tag='acc',
                                             name='acc')
                            nc.vector.tensor_scalar(
                                acc[:], g2[0][:, jtl, 0:CIN],
                                w4[:, jt, k, 0].unsqueeze(1), None, ALU.mult)
                            for (gt, sl, t) in ((g2[0], 1, 1), (g2[1], 0, 2),
                                                (g2[1], 1, 3)):
                                nc.vector.scalar_tensor_tensor(
                                    acc[:], gt[:, jtl, sl * CIN:(sl + 1) * CIN],
                                    w4[:, jt, k, t].unsqueeze(1), acc[:],
                                    ALU.mult, ALU.add)
                            ps = ps_small.tile([128, 128], BF16, tag='psb',
                                               name='ps')
                            nc.tensor.transpose(ps[:], acc[:], identb[:])
                            nc.scalar.activation(
                                val[:, 128 * jtl:128 * (jtl + 1)], ps[:],
                                ACTF.Copy)
                        vals.append(val)

                    if phase != 3:
                        continue
                    for jc in range(JH // 512):
                        for oh in range(2):
                            ps = ps_conv.tile([128, 512], F32, tag='ps_conv')
                            for k in range(NK):
                                nc.tensor.matmul(
                                    ps[:],
                                    lhsT=wdefT[:, k, 128 * oh:128 * (oh + 1)],
                                    rhs=vals[k][:, 512 * jc:512 * (jc + 1)],
                                    start=(k == 0), stop=(k == NK - 1))
                            yo = opool.tile([128, 512], F32, tag='yo')
                            nc.scalar.activation(
                                yo[:], ps[:], ACTF.Relu,
                                bias=bdef[:, oh:oh + 1], scale=1.0)
                            nc.sync.dma_start(
                                y_d.ap()[128 * oh:128 * (oh + 1),
                                         j0 + 512 * jc:j0 + 512 * (jc + 1)],
                                yo[:])


def _host_prep(x, w_off, b_off, w_def, b_def):
    """Build per-core input maps."""
    x = np.asarray(x, np.float32)
    w_off = np.asarray(w_off, np.float32)
    b_off = np.asarray(b_off, np.float32)
    w_def = np.asarray(w_def, np.float32)
    b_def = np.asarray(b_def, np.float32)

    woffT = np.stack([w_off[:, :, s // 3, s % 3].T for s in range(9)])
    woffT = _to_bf16(np.ascontiguousarray(woffT, np.float32))  # [9, 128, 18]
    wdefT = np.stack([w_def[:, :, s // 3, s % 3].T for s in range(9)])
    wdefT = _to_bf16(np.ascontiguousarray(wdefT))             # [9, 128, 256]
    bdef2 = np.ascontiguousarray(b_def.reshape(2, 128).T)     # [128, 2]
    ident = np.eye(128, dtype=np.float32)

    jp = np.arange(128)[:, None, None]
    jt = np.arange(NJT)[None, :, None]
    kk = np.arange(NK)[None, None, :]
    j = jt * 128 + jp
    ky = np.array(KY, np.float32)[kk]
    kx = np.array(KX, np.float32)[kk]
    hgk = (j // 64).astype(np.float32) + ky
    wgk = (j % 64).astype(np.float32) + kx
    hgk = np.ascontiguousarray(np.broadcast_to(hgk, (128, NJT, NK)), np.float32)
    wgk = np.ascontiguousarray(np.broadcast_to(wgk, (128, NJT, NK)), np.float32)

    xp = np.pad(x, ((0, 0), (0, 0), (1, 1), (1, 1))).reshape(B, CIN, 66 * 66)

    shared = {
        'woffT': woffT,
        'boff': np.ascontiguousarray(b_off.reshape(18, 1)),
        'wdefT': wdefT,
        'bdef': bdef2,
        'ident': ident,
        'hgk': hgk,
        'wgk': wgk,
    }
    in_maps = []
    for b in range(B):
        m = dict(shared)
        m['xp'] = np.ascontiguousarray(xp[b])
        in_maps.append(m)
    return in_maps


def _to_bf16(a):
    import ml_dtypes
    return a.astype(ml_dtypes.bfloat16)


LAST_RESULTS = None


def _ensure_trace_support():
    """Register the NTFF profile hook that the slim agent image lacks, and
    stub out the artifact upload. Only used when KBENCH_TRACE is set."""
    import contextlib
    import ctypes
    import types

    import concourse.bass_utils as bu
    bu.upload_artifacts = lambda tmpdir: tmpdir

    if 'antenv.axon_hooks' in sys.modules:
        return
    so_path = '/opt/axon/libaxon_pjrt.so'
    if not os.path.exists(so_path):
        return
    lib = ctypes.CDLL(so_path)
    if not hasattr(lib, 'axon_start_nrt_profile'):
        return
    lib.axon_start_nrt_profile.argtypes = [
        ctypes.POINTER(ctypes.c_int64), ctypes.c_size_t]
    lib.axon_start_nrt_profile.restype = ctypes.c_int64
    lib.axon_stop_nrt_profile.argtypes = [ctypes.c_char_p]
    lib.axon_stop_nrt_profile.restype = ctypes.c_int64

    @contextlib.contextmanager
    def _hook(output_dir, device_ids):
        import jax
        jax.devices()
        if device_ids:
            ids = (ctypes.c_int64 * len(device_ids))(*device_ids)
            rc = lib.axon_start_nrt_profile(ids, len(device_ids))
        else:
            rc = lib.axon_start_nrt_profile(None, 0)
        if rc != 0:
            raise RuntimeError(f'axon_start_nrt_profile rc={rc}')
        try:
            yield
        finally:
            n = lib.axon_stop_nrt_profile(str(output_dir).encode())
            print(f'profile: {n} file(s) written to {output_dir}',
                  file=sys.stderr)

    mod = types.ModuleType('antenv.axon_hooks')
    mod.get_axon_ntff_profile_hook = lambda: _hook
    mod.set_axon_ntff_profile_hook = lambda h: None
    sys.modules['antenv.axon_hooks'] = mod


def kernel(x, w_off, b_off, w_def, b_def):
    global LAST_RESULTS
    if 'nc' not in _CACHE:
        _CACHE['nc'] = _build_program(
            phase=int(os.environ.get('KBENCH_PHASE', '3')))
    nc = _CACHE['nc']
    in_maps = _host_prep(x, w_off, b_off, w_def, b_def)
    trace = bool(os.environ.get('KBENCH_TRACE'))
    if trace:
        _ensure_trace_support()
    res = run_bass_kernel_spmd(
        nc, in_maps, core_ids=list(range(B)),
        trace=trace,
    )
    LAST_RESULTS = res
    out = np.stack([res.results[b]['y'].reshape(COUT, H, W) for b in range(B)])
    return out.astype(np.float32)



# revision 3
# speedup vs baseline: 1.9527x; 1.9527x over previous
"""Trainium2 Bass kernel for nn_DeformableBlock (deformable conv v1 block).

Contract: kernel(**inputs) takes FULL unsharded inputs, returns FULL output.
Sharding: data-parallel over batch (B=8 -> 8 NeuronCores, 1 batch each),
weights replicated.

Per-core algorithm (one batch, Cin=128, Cout=256, H=W=64), v2:
  1. offset conv (3x3, pad 1) as 9 shifted bf16 matmuls -> offset [18, 4096]
  2. PE-transpose offsets to pixel-major; DVE computes ONE gather index per
     (pixel, k) plus 4 bilinear tap weights w4. Taps outside the image are
     zero-weighted; gather rows are clipped into a zero-padded layout.
  3. xT2 "paired-row" DRAM layout: row r = yy*65+xx (65x65 padded grid)
     holds [x[yy-1, xx-1, :], x[yy, xx-1, :]] (256 bf16). One gather elem =
     rows (r, r+1) = 512 bf16 = the full 2x2 bilinear window for all 128
     channels -> ONE descriptor per (pixel, k): 18 gathers x 2048 idx.
  4. tap weighting: one DVE tensor_tensor per (k, half) with a stride-0
     broadcast AP on w4 (weights repeat across the 128-channel free dim);
     the 4-tap sum is folded into PE via 4 accumulating identity-matmuls
     that also transpose pixel-major -> channel-major; ACT evacuates PSUM.
  5. main conv: out[o, j] = sum_k W_k[o,:] @ val_k  (bf16 matmuls, fp32 PSUM)
  6. bias + ReLU on ScalarE, DMA out [256, 4096] f32
"""
import os
import sys
import numpy as np

try:
    import concourse.bass as bass
except ImportError:  # pragma: no cover
    sys.path.insert(0, '/opt/trn_rl_repo')
    import concourse.bass as bass
import concourse.bacc as bacc

import concourse.mybir as mybir
import concourse.tile as tile
from concourse import library_config
from concourse.bass_utils import run_bass_kernel_spmd

F32 = mybir.dt.float32
BF16 = mybir.dt.bfloat16
I32 = mybir.dt.int32
I16 = mybir.dt.int16
ALU = mybir.AluOpType
ACTF = mybir.ActivationFunctionType

B, CIN, COUT, H, W = 8, 128, 256, 64, 64
HW = H * W          # 4096
NJT = HW // 128     # 32 pixel-major tiles
NK = 9
KY = [(-1), (-1), (-1), 0, 0, 0, 1, 1, 1]
KX = [(-1), 0, 1, (-1), 0, 1, (-1), 0, 1]
NHALF = 2
JH = HW // NHALF    # 2048 pixels per half
NPG = 66 * 66       # padded grid pixels
R2 = 65 * 65 + 1    # xT2 rows (incl 1 zero row read by r+1 at the corner)
NPT = 35            # ceil(4356 / 128) transpose tiles of the padded grid

_CACHE = {}


def _split_multiwaits(nc, max_waits=1, kinds=None):
    """walrus CoreV3 codegen rejects control instructions carrying more
    than one sem-wait; split the excess into a chain of same-engine
    drains placed directly before the offender."""
    if kinds is None:
        kinds = (mybir.InstDrain,)
    n_split = 0
    for fn in nc.m.functions:
        for bb in fn.blocks:
            insts = list(bb.instructions)
            new = []
            changed = False
            for inst in insts:
                si = inst.sync_info
                if (isinstance(inst, kinds) and si is not None
                        and len(si.on_wait) > max_waits):
                    waits = list(si.on_wait)
                    pre, rest = waits[:-max_waits], waits[-max_waits:]
                    for i in range(0, len(pre), max_waits):
                        chunk = pre[i:i + max_waits]
                        d = mybir.InstDrain(
                            name=f"{inst.name}-wsplit{i}",
                            engine=inst.engine,
                            ins=[], outs=[],
                            sync_info=mybir.SyncInfo(
                                on_wait=chunk, on_update=[]),
                        )
                        new.append(d)
                        n_split += 1
                    inst.sync_info = mybir.SyncInfo(
                        on_wait=rest, on_update=list(si.on_update))
                    changed = True
                new.append(inst)
            if changed:
                bb.instructions = new
    return n_split


def _build_program(phase=3):
    nc = bacc.Bacc('TRN2', target_bir_lowering=False, debug=False,
                   enable_asserts=False, num_devices=B)

    # ---- DRAM I/O ----
    xp_d = nc.dram_tensor('xp', [CIN, NPG], F32, kind='ExternalInput')
    woffT_d = nc.dram_tensor('woffT', [9, CIN, 18], BF16, kind='ExternalInput')
    boff_d = nc.dram_tensor('boff', [18, 1], F32, kind='ExternalInput')
    wdefT_d = nc.dram_tensor('wdefT', [NK, CIN, COUT], BF16, kind='ExternalInput')
    bdef_d = nc.dram_tensor('bdef', [128, 2], F32, kind='ExternalInput')
    ident_d = nc.dram_tensor('ident', [128, 128], F32, kind='ExternalInput')
    hg1k_d = nc.dram_tensor('hg1k', [128, NJT, NK], F32, kind='ExternalInput')
    wg1k_d = nc.dram_tensor('wg1k', [128, NJT, NK], F32, kind='ExternalInput')
    y_d = nc.dram_tensor('y', [COUT, HW], F32, kind='ExternalOutput')

    # DRAM scratch
    xpT_d = nc.dram_tensor('xpT_scratch', [NPT * 128, CIN], BF16, kind='Internal')
    xT2_d = nc.dram_tensor('xT2_scratch', [R2, 2 * CIN], BF16, kind='Internal')
    idxnat_d = nc.dram_tensor('idxnat', [NK, HW], I16, kind='Internal')

    with tile.TileContext(nc) as tc:
        with tc.tile_pool(name='const', bufs=1) as cpool:
            # persistent small tensors
            wdefT = cpool.tile([CIN, NK, COUT], BF16, tag='wdefT')
            nc.sync.dma_start(
                wdefT[:], wdefT_d.ap().rearrange('k c o -> c k o'))
            bdef = cpool.tile([128, 2], F32, tag='bdef')
            nc.sync.dma_start(bdef[:], bdef_d.ap())
            idxwr = cpool.tile([128, NK, NHALF, JH // 16], I16, tag='idxwr')
            w4b = cpool.tile([128, NJT, NK, 4], BF16, tag='w4b')
            identb = cpool.tile([128, 128], BF16, tag='identb')

            # ======== phase 1: offsets, indices, weights, xT2 ========
            with (
                tc.tile_pool(name='ps_small', bufs=2, space='PSUM') as ps_small,
                tc.tile_pool(name='p1', bufs=1) as apool,
            ):
                x_sb = apool.tile([CIN, NPG], F32, tag='x_sb')
                nc.sync.dma_start(x_sb[:], xp_d.ap())
                xbf = apool.tile([CIN, NPG], BF16, tag='xbf')
                nc.vector.tensor_copy(xbf[:], x_sb[:])
                ident = apool.tile([128, 128], F32, tag='ident')
                nc.sync.dma_start(ident[:], ident_d.ap())
                woffT = apool.tile([CIN, 9, 18], BF16, tag='woffT')
                nc.sync.dma_start(
                    woffT[:], woffT_d.ap().rearrange('s c o -> c s o'))
                boff = apool.tile([18, 1], F32, tag='boff')
                nc.sync.dma_start(boff[:], boff_d.ap())
                hg1k = apool.tile([128, NJT, NK], F32, tag='hg1k')
                nc.sync.dma_start(hg1k[:], hg1k_d.ap())
                wg1k = apool.tile([128, NJT, NK], F32, tag='wg1k')
                nc.sync.dma_start(wg1k[:], wg1k_d.ap())
                nc.vector.tensor_copy(identb[:], ident[:])

                # ---------- offset conv: off [18, 4096] f32 ----------
                off_sb = apool.tile([18, HW], F32, tag='off_sb')
                chunks = [(1 + 7 * i, 7) for i in range(9)] + [(64, 1)]
                for r0, nr in chunks:
                    nfree = 66 * (nr - 1) + 64
                    ps = ps_small.tile([18, 512], F32, tag='ps', name='ps')
                    for s in range(9):
                        dh, dw = s // 3, s % 3
                        beg = r0 * 66 + 1 + (dh - 1) * 66 + (dw - 1)
                        rhs = bass.AP(
                            tensor=xbf[:].tensor,
                            offset=xbf[:].offset + beg,
                            ap=[list(xbf[:].ap[0]), [1, nfree]],
                        )
                        nc.tensor.matmul(
                            ps[:, :nfree], lhsT=woffT[:, s, :], rhs=rhs,
                            start=(s == 0), stop=(s == 8))
                    src_in = bass.AP(
                        tensor=ps[:].tensor, offset=ps[:].offset,
                        ap=[list(ps[:].ap[0]), [66, nr], [1, 64]])
                    nc.scalar.activation(
                        off_sb[:, 64 * (r0 - 1):64 * (r0 - 1 + nr)], src_in,
                        ACTF.Identity, bias=boff[:], scale=1.0)

                # ---------- transpose offsets to pixel-major ----------
                offT = apool.tile([128, NJT, 18], F32, tag='offT')
                for jt in range(NJT):
                    ps = ps_small.tile([128, 18], F32, tag='ps')
                    nc.tensor.transpose(
                        ps[:], off_sb[:, 128 * jt:128 * (jt + 1)],
                        ident[:18, :18])
                    nc.vector.tensor_copy(offT[:, jt, :], ps[:])

                # ---------- xT2 paired-row build ----------
                # transpose the whole padded grid to pixel-major in DRAM,
                # then two strided DRAM->DRAM copies interleave row pairs.
                xpT = apool.tile([128, NPT, CIN], BF16, tag='xpT')
                for t in range(NPT):
                    c0 = 128 * t
                    ncol = min(128, NPG - c0)
                    psb = ps_small.tile([128, 128], BF16, tag='psb', name='ps')
                    nc.tensor.transpose(
                        psb[:ncol, :], xbf[:, c0:c0 + ncol], identb[:])
                    nc.vector.tensor_copy(xpT[:ncol, t, :], psb[:ncol, :])
                nc.sync.dma_start(
                    xpT_d.ap().rearrange('(t p) c -> p t c', p=128), xpT[:])
                for dst_half, src_row in ((0, 0), (1, 1)):
                    dst = bass.AP(
                        tensor=xT2_d, offset=dst_half * CIN,
                        ap=[[65 * 2 * CIN, 65], [2 * CIN, 65], [1, CIN]])
                    src = bass.AP(
                        tensor=xpT_d, offset=src_row * 66 * CIN,
                        ap=[[66 * CIN, 65], [CIN, 65], [1, CIN]])
                    nc.sync.dma_start(dst, src)
                zpad = apool.tile([1, 2 * CIN], BF16, tag='zpad')
                nc.vector.memset(zpad[:], 0)
                nc.sync.dma_start(
                    bass.AP(tensor=xT2_d, offset=(R2 - 1) * 2 * CIN,
                            ap=[[2 * CIN, 1], [1, 2 * CIN]]),
                    zpad[:])

                # ---------- index/weight arithmetic (pixel-major) ----------
                sh = [128, NJT, NK]

                def T(tag, dt=F32):
                    return apool.tile(sh, dt, tag=tag, name=tag)

                dyx = offT[:].rearrange('p jt (k two) -> p jt k two', two=2)
                dy = dyx[:, :, :, 0]
                dx = dyx[:, :, :, 1]

                ti = apool.tile(sh, I32, tag='ti')
                fdy, fdx = T('fdy'), T('fdx')
                tmp1, tmp2 = T('tmp1'), T('tmp2')
                # floor(dy)
                nc.vector.tensor_copy(ti[:], dy)
                nc.vector.tensor_copy(fdy[:], ti[:])
                nc.vector.tensor_tensor(tmp1[:], fdy[:], dy, ALU.is_gt)
                nc.vector.tensor_tensor(fdy[:], fdy[:], tmp1[:], ALU.subtract)
                # floor(dx)
                nc.vector.tensor_copy(ti[:], dx)
                nc.vector.tensor_copy(fdx[:], ti[:])
                nc.vector.tensor_tensor(tmp1[:], fdx[:], dx, ALU.is_gt)
                nc.vector.tensor_tensor(fdx[:], fdx[:], tmp1[:], ALU.subtract)

                ly, lx = T('ly'), T('lx')
                nc.vector.tensor_tensor(ly[:], dy, fdy[:], ALU.subtract)
                nc.vector.tensor_tensor(lx[:], dx, fdx[:], ALU.subtract)

                # ry = h + ky + fdy + 1 (base row of the gathered pair);
                # y0 = ry - 1, y1 = ry
                ry, rx = T('ry'), T('rx')
                nc.vector.tensor_tensor(ry[:], hg1k[:], fdy[:], ALU.add)
                nc.vector.tensor_tensor(rx[:], wg1k[:], fdx[:], ALU.add)

                y0, x0 = T('y0'), T('x0')
                nc.vector.tensor_scalar(y0[:], ry[:], 1.0, None, ALU.subtract)
                nc.vector.tensor_scalar(x0[:], rx[:], 1.0, None, ALU.subtract)

                vy0, vy1, vx0, vx1 = T('vy0'), T('vy1'), T('vx0'), T('vx1')
                nc.vector.tensor_scalar(tmp1[:], y0[:], 63.0, 0.0, ALU.min, ALU.max)
                nc.vector.tensor_tensor(vy0[:], y0[:], tmp1[:], ALU.is_equal)
                nc.vector.tensor_scalar(tmp1[:], ry[:], 63.0, 0.0, ALU.min, ALU.max)
                nc.vector.tensor_tensor(vy1[:], ry[:], tmp1[:], ALU.is_equal)
                nc.vector.tensor_scalar(tmp1[:], x0[:], 63.0, 0.0, ALU.min, ALU.max)
                nc.vector.tensor_tensor(vx0[:], x0[:], tmp1[:], ALU.is_equal)
                nc.vector.tensor_scalar(tmp1[:], rx[:], 63.0, 0.0, ALU.min, ALU.max)
                nc.vector.tensor_tensor(vx1[:], rx[:], tmp1[:], ALU.is_equal)

                # tap weights: wy0 = (1-ly)*vy0, wy1 = ly*vy1 (x analogous)
                wy0, wy1, wx0, wx1 = T('wy0'), T('wy1'), T('wx0'), T('wx1')
                nc.vector.tensor_tensor(tmp1[:], ly[:], vy0[:], ALU.mult)
                nc.vector.tensor_tensor(wy0[:], vy0[:], tmp1[:], ALU.subtract)
                nc.vector.tensor_tensor(wy1[:], ly[:], vy1[:], ALU.mult)
                nc.vector.tensor_tensor(tmp1[:], lx[:], vx0[:], ALU.mult)
                nc.vector.tensor_tensor(wx0[:], vx0[:], tmp1[:], ALU.subtract)
                nc.vector.tensor_tensor(wx1[:], lx[:], vx1[:], ALU.mult)

                # w4 (tap order matches the gathered elem layout:
                # [y0x0, y1x0, y0x1, y1x1])
                w4 = apool.tile([128, NJT, NK, 4], F32, tag='w4')
                nc.vector.tensor_tensor(w4[:, :, :, 0], wy0[:], wx0[:], ALU.mult)
                nc.vector.tensor_tensor(w4[:, :, :, 1], wy1[:], wx0[:], ALU.mult)
                nc.vector.tensor_tensor(w4[:, :, :, 2], wy0[:], wx1[:], ALU.mult)
                nc.vector.tensor_tensor(w4[:, :, :, 3], wy1[:], wx1[:], ALU.mult)
                nc.vector.tensor_copy(w4b[:], w4[:])

                # gather index = clip(ry,0,64)*65 + clip(rx,0,64)
                idxf = T('idxf')
                nc.vector.tensor_scalar(tmp1[:], ry[:], 64.0, 0.0, ALU.min, ALU.max)
                nc.vector.tensor_scalar(tmp2[:], rx[:], 64.0, 0.0, ALU.min, ALU.max)
                nc.vector.tensor_scalar(tmp1[:], tmp1[:], 65.0, None, ALU.mult)
                nc.vector.tensor_tensor(idxf[:], tmp1[:], tmp2[:], ALU.add)

                # ---------- un-transpose idx to j-natural via PE ----------
                idx_tr = apool.tile([NJT, NK, 128], I16, tag='idx_tr')
                for k in range(NK):
                    ps = ps_small.tile([NJT, 128], F32, tag='ps')
                    nc.tensor.transpose(ps[:], idxf[:, :, k], ident[:])
                    nc.vector.tensor_copy(idx_tr[:, k, :], ps[:])

                # ---------- reformat via DRAM ----------
                nc.sync.dma_start(
                    idxnat_d.ap().rearrange('k (jt jp) -> jt k jp', jp=128),
                    idx_tr[:])

                # wrapped idx: [128, NK, NHALF, JH/16] int16 (i%16 partition
                # wrap via HWDGE xbar transpose, then replicate to 8 groups)
                for k in range(NK):
                    for half in range(NHALF):
                        src = idxnat_d.ap()[k, half * JH:(half + 1) * JH]
                        nc.sync.dma_start_transpose(
                            idxwr[0:16, k, half, :],
                            src.rearrange('(f p) -> f p', p=16))
                for g in range(1, 8):
                    nc.sync.dma_start(idxwr[16 * g:16 * (g + 1)], idxwr[0:16])

            if phase != 1:
                _phase2(nc, tc, wdefT, bdef, idxwr, w4b, identb,
                        xT2_d, y_d, phase)
            if phase != 3:
                with tc.tile_pool(name='zz', bufs=1) as zp:
                    zt = zp.tile([128, HW], F32, tag='zt')
                    nc.vector.memset(zt[:], 0)
                    for oh in range(2):
                        nc.sync.dma_start(
                            y_d.ap()[128 * oh:128 * (oh + 1), :], zt[:])

    nc.finalize()
    _split_multiwaits(nc)
    return nc


def _phase2(nc, tc, wdefT, bdef, idxwr, w4b, identb, xT2_d, y_d, phase=3):
    # ======== phase 2: gather + weighting + conv ========
    with (
        tc.tile_pool(name='ps_tr', bufs=4, space='PSUM') as ps_tr,
        tc.tile_pool(name='ps_conv', bufs=2, space='PSUM') as ps_conv,
        tc.tile_pool(name='gath', bufs=2) as gpool,
        tc.tile_pool(name='prodp', bufs=2) as tpool,
        tc.tile_pool(name='val', bufs=2) as vpool,
        tc.tile_pool(name='outp', bufs=2) as opool,
    ):
        w4b_ap = w4b[:]
        for half in range(NHALF):
            j0 = half * JH
            valT = vpool.tile([128, NK, JH // 128, 128], BF16, tag='valT')
            for k in range(NK):
                # one descriptor per (pixel, k): elem = xT2 rows (r, r+1)
                # = 512 bf16 = the 2x2 bilinear window over all channels
                G = gpool.tile([128, JH // 128, 4 * CIN], BF16, tag='G',
                               name='G')
                in_ap = bass.AP(
                    tensor=xT2_d, offset=0,
                    ap=[[2 * CIN, R2 - 1], [1, 4 * CIN]],
                )
                nc.gpsimd.dma_gather(
                    out_ap=G[:],
                    in_ap=in_ap,
                    idxs_ap=idxwr[:, k, half, :],
                    num_idxs=JH,
                    num_idxs_reg=JH,
                    elem_size=4 * CIN,
                    elem_step=2 * CIN,
                    transpose=False,
                    single_packet=False,
                )
                if phase == 21:
                    continue
                # tap weighting: one DVE op per (k, half); the weight AP
                # broadcasts each w4 value across the 128-channel free dim
                prod = tpool.tile([128, JH // 128, 4, CIN], BF16, tag='prod',
                                  name='prod')
                wap = bass.AP(
                    tensor=w4b_ap.tensor,
                    offset=(w4b_ap.offset + (half * (JH // 128)) * (NK * 4)
                            + k * 4),
                    ap=[list(w4b_ap.ap[0]), [NK * 4, JH // 128], [1, 4],
                        [0, CIN]],
                )
                gv = G[:].rearrange('p j (t c) -> p j t c', t=4)
                nc.vector.tensor_tensor(prod[:], gv, wap, ALU.mult)
                # 4-tap sum + transpose to channel-major, fused on PE
                for jtl in range(JH // 128):
                    ps = ps_tr.tile([128, 128], F32, tag='pst', name='ps')
                    for t in range(4):
                        nc.tensor.matmul(
                            ps[:], lhsT=prod[:, jtl, t, :], rhs=identb[:],
                            start=(t == 0), stop=(t == 3))
                    nc.scalar.activation(valT[:, k, jtl, :], ps[:], ACTF.Copy)

            if phase != 3:
                continue
            valT_ap = valT[:]
            for jc in range(JH // 512):
                for oh in range(2):
                    psc = ps_conv.tile([128, 512], F32, tag='ps_conv')
                    for k in range(NK):
                        rhs = bass.AP(
                            tensor=valT_ap.tensor,
                            offset=valT_ap.offset + k * JH + jc * 512,
                            ap=[list(valT_ap.ap[0]), [1, 512]],
                        )
                        nc.tensor.matmul(
                            psc[:],
                            lhsT=wdefT[:, k, 128 * oh:128 * (oh + 1)],
                            rhs=rhs,
                            start=(k == 0), stop=(k == NK - 1))
                    yo = opool.tile([128, 512], F32, tag='yo')
                    nc.scalar.activation(
                        yo[:], psc[:], ACTF.Relu,
                        bias=bdef[:, oh:oh + 1], scale=1.0)
                    nc.sync.dma_start(
                        y_d.ap()[128 * oh:128 * (oh + 1),
                                 j0 + 512 * jc:j0 + 512 * (jc + 1)],
                        yo[:])


def _host_prep(x, w_off, b_off, w_def, b_def):
    """Build per-core input maps."""
    x = np.asarray(x, np.float32)
    w_off = np.asarray(w_off, np.float32)
    b_off = np.asarray(b_off, np.float32)
    w_def = np.asarray(w_def, np.float32)
    b_def = np.asarray(b_def, np.float32)

    woffT = np.stack([w_off[:, :, s // 3, s % 3].T for s in range(9)])
    woffT = _to_bf16(np.ascontiguousarray(woffT, np.float32))  # [9, 128, 18]
    wdefT = np.stack([w_def[:, :, s // 3, s % 3].T for s in range(9)])
    wdefT = _to_bf16(np.ascontiguousarray(wdefT))             # [9, 128, 256]
    bdef2 = np.ascontiguousarray(b_def.reshape(2, 128).T)     # [128, 2]
    ident = np.eye(128, dtype=np.float32)

    jp = np.arange(128)[:, None, None]
    jt = np.arange(NJT)[None, :, None]
    kk = np.arange(NK)[None, None, :]
    j = jt * 128 + jp
    ky = np.array(KY, np.float32)[kk]
    kx = np.array(KX, np.float32)[kk]
    hg1k = (j // 64).astype(np.float32) + ky + 1.0
    wg1k = (j % 64).astype(np.float32) + kx + 1.0
    hg1k = np.ascontiguousarray(np.broadcast_to(hg1k, (128, NJT, NK)), np.float32)
    wg1k = np.ascontiguousarray(np.broadcast_to(wg1k, (128, NJT, NK)), np.float32)

    xp = np.pad(x, ((0, 0), (0, 0), (1, 1), (1, 1))).reshape(B, CIN, NPG)

    shared = {
        'woffT': woffT,
        'boff': np.ascontiguousarray(b_off.reshape(18, 1)),
        'wdefT': wdefT,
        'bdef': bdef2,
        'ident': ident,
        'hg1k': hg1k,
        'wg1k': wg1k,
    }
    in_maps = []
    for b in range(B):
        m = dict(shared)
        m['xp'] = np.ascontiguousarray(xp[b])
        in_maps.append(m)
    return in_maps


def _to_bf16(a):
    import ml_dtypes
    return a.astype(ml_dtypes.bfloat16)


LAST_RESULTS = None


def _ensure_trace_support():
    """Register the NTFF profile hook that the slim agent image lacks, and
    stub out the artifact upload. Only used when KBENCH_TRACE is set."""
    import contextlib
    import ctypes
    import types

    import concourse.bass_utils as bu
    bu.upload_artifacts = lambda tmpdir: tmpdir

    if 'antenv.axon_hooks' in sys.modules:
        return
    so_path = '/opt/axon/libaxon_pjrt.so'
    if not os.path.exists(so_path):
        return
    lib = ctypes.CDLL(so_path)
    if not hasattr(lib, 'axon_start_nrt_profile'):
        return
    lib.axon_start_nrt_profile.argtypes = [
        ctypes.POINTER(ctypes.c_int64), ctypes.c_size_t]
    lib.axon_start_nrt_profile.restype = ctypes.c_int64
    lib.axon_stop_nrt_profile.argtypes = [ctypes.c_char_p]
    lib.axon_stop_nrt_profile.restype = ctypes.c_int64

    @contextlib.contextmanager
    def _hook(output_dir, device_ids):
        import jax
        jax.devices()
        if device_ids:
            ids = (ctypes.c_int64 * len(device_ids))(*device_ids)
            rc = lib.axon_start_nrt_profile(ids, len(device_ids))
        else:
            rc = lib.axon_start_nrt_profile(None, 0)
        if rc != 0:
            raise RuntimeError(f'axon_start_nrt_profile rc={rc}')
        try:
            yield
        finally:
            n = lib.axon_stop_nrt_profile(str(output_dir).encode())
            print(f'profile: {n} file(s) written to {output_dir}',
                  file=sys.stderr)

    mod = types.ModuleType('antenv.axon_hooks')
    mod.get_axon_ntff_profile_hook = lambda: _hook
    mod.set_axon_ntff_profile_hook = lambda h: None
    sys.modules['antenv.axon_hooks'] = mod


def kernel(x, w_off, b_off, w_def, b_def):
    global LAST_RESULTS
    if 'nc' not in _CACHE:
        _CACHE['nc'] = _build_program(
            phase=int(os.environ.get('KBENCH_PHASE', '3')))
    nc = _CACHE['nc']
    in_maps = _host_prep(x, w_off, b_off, w_def, b_def)
    trace = bool(os.environ.get('KBENCH_TRACE'))
    if trace:
        _ensure_trace_support()
    res = run_bass_kernel_spmd(
        nc, in_maps, core_ids=list(range(B)),
        trace=trace,
    )
    LAST_RESULTS = res
    out = np.stack([res.results[b]['y'].reshape(COUT, H, W) for b in range(B)])
    return out.astype(np.float32)


# revision 14
# speedup vs baseline: 2.6488x; 1.3565x over previous
"""Trainium2 Bass kernel for nn_DeformableBlock (deformable conv v1 block).

Contract: kernel(**inputs) takes FULL unsharded inputs, returns FULL output.
Sharding: data-parallel over batch (B=8 -> 8 NeuronCores, 1 batch each),
weights replicated.

Per-core algorithm (one batch, Cin=128, Cout=256, H=W=64), v2:
  1. offset conv (3x3, pad 1) as 9 shifted bf16 matmuls -> offset [18, 4096]
  2. PE-transpose offsets to pixel-major; DVE computes ONE gather index per
     (pixel, k) plus 4 bilinear tap weights w4. Taps outside the image are
     zero-weighted; gather rows are clipped into a zero-padded layout.
  3. xT2 "paired-row" DRAM layout: row r = yy*65+xx (65x65 padded grid)
     holds [x[yy-1, xx-1, :], x[yy, xx-1, :]] (256 bf16). One gather elem =
     rows (r, r+1) = 512 bf16 = the full 2x2 bilinear window for all 128
     channels -> ONE descriptor per (pixel, k): 18 gathers x 2048 idx.
  4. tap weighting: one DVE tensor_tensor per (k, half) with a stride-0
     broadcast AP on w4 (weights repeat across the 128-channel free dim);
     the 4-tap sum is folded into PE via 4 accumulating identity-matmuls
     that also transpose pixel-major -> channel-major; ACT evacuates PSUM.
  5. main conv: out[o, j] = sum_k W_k[o,:] @ val_k  (bf16 matmuls, fp32 PSUM)
  6. bias + ReLU on ScalarE, DMA out [256, 4096] f32
"""
import os
import sys
import numpy as np

try:
    import concourse.bass as bass
except ImportError:  # pragma: no cover
    sys.path.insert(0, '/opt/trn_rl_repo')
    import concourse.bass as bass
import concourse.bacc as bacc

import concourse.mybir as mybir
import concourse.tile as tile
from concourse import library_config
from concourse.bass_utils import run_bass_kernel_spmd

F32 = mybir.dt.float32
BF16 = mybir.dt.bfloat16
I32 = mybir.dt.int32
I16 = mybir.dt.int16
ALU = mybir.AluOpType
ACTF = mybir.ActivationFunctionType

B, CIN, COUT, H, W = 8, 128, 256, 64, 64
HW = H * W          # 4096
NJT = HW // 128     # 32 pixel-major tiles
NK = 9
KY = [(-1), (-1), (-1), 0, 0, 0, 1, 1, 1]
KX = [(-1), 0, 1, (-1), 0, 1, (-1), 0, 1]
NHALF = 2
JH = HW // NHALF    # 2048 pixels per half
NPG = 66 * 66       # padded grid pixels
R2 = 65 * 65 + 1    # xT2 rows (incl 1 zero row read by r+1 at the corner)
NPT = 35            # ceil(4356 / 128) transpose tiles of the padded grid

_CACHE = {}


def _split_multiwaits(nc, max_waits=1, kinds=None):
    """walrus CoreV3 codegen rejects control instructions carrying more
    than one sem-wait; split the excess into a chain of same-engine
    drains placed directly before the offender."""
    if kinds is None:
        kinds = (mybir.InstDrain,)
    n_split = 0
    for fn in nc.m.functions:
        for bb in fn.blocks:
            insts = list(bb.instructions)
            new = []
            changed = False
            for inst in insts:
                si = inst.sync_info
                if (isinstance(inst, kinds) and si is not None
                        and len(si.on_wait) > max_waits):
                    waits = list(si.on_wait)
                    pre, rest = waits[:-max_waits], waits[-max_waits:]
                    for i in range(0, len(pre), max_waits):
                        chunk = pre[i:i + max_waits]
                        d = mybir.InstDrain(
                            name=f"{inst.name}-wsplit{i}",
                            engine=inst.engine,
                            ins=[], outs=[],
                            sync_info=mybir.SyncInfo(
                                on_wait=chunk, on_update=[]),
                        )
                        new.append(d)
                        n_split += 1
                    inst.sync_info = mybir.SyncInfo(
                        on_wait=rest, on_update=list(si.on_update))
                    changed = True
                new.append(inst)
            if changed:
                bb.instructions = new
    return n_split


def _build_program(phase=3):
    nc = bacc.Bacc('TRN2', target_bir_lowering=False, debug=False,
                   enable_asserts=False, num_devices=B, num_swdge_queues=4)

    # ---- DRAM I/O ----
    xp_d = nc.dram_tensor('xp', [CIN, NPG], F32, kind='ExternalInput')
    woffT_d = nc.dram_tensor('woffT', [9, CIN, 18], BF16, kind='ExternalInput')
    boff_d = nc.dram_tensor('boff', [18, 1], F32, kind='ExternalInput')
    wdefT_d = nc.dram_tensor('wdefT', [NK, CIN, COUT], BF16, kind='ExternalInput')
    bdef_d = nc.dram_tensor('bdef', [128, 2], F32, kind='ExternalInput')
    ident_d = nc.dram_tensor('ident', [128, 128], F32, kind='ExternalInput')
    hg1k_d = nc.dram_tensor('hg1k', [128, NJT, NK], F32, kind='ExternalInput')
    wg1k_d = nc.dram_tensor('wg1k', [128, NJT, NK], F32, kind='ExternalInput')
    y_d = nc.dram_tensor('y', [COUT, HW], F32, kind='ExternalOutput')

    # DRAM scratch
    xT2_d = nc.dram_tensor('xT2_scratch', [R2, 2 * CIN], BF16, kind='Internal')
    idxnat_d = nc.dram_tensor('idxnat', [NK, HW], I16, kind='Internal')

    with tile.TileContext(nc) as tc:
        with tc.tile_pool(name='const', bufs=1) as cpool:
            # persistent small tensors
            wdefT = cpool.tile([CIN, NK, COUT], BF16, tag='wdefT')
            nc.sync.dma_start(
                wdefT[:], wdefT_d.ap().rearrange('k c o -> c k o'))
            bdef = cpool.tile([128, 2], F32, tag='bdef')
            nc.sync.dma_start(bdef[:], bdef_d.ap())
            idxwr = cpool.tile([128, NK, NHALF, JH // 16], I16, tag='idxwr')
            w4b = cpool.tile([128, NJT, NK, 4], BF16, tag='w4b')
            identb = cpool.tile([128, 128], BF16, tag='identb')

            # ======== phase 1: offsets, indices, weights, xT2 ========
            with (
                tc.tile_pool(name='ps_small', bufs=2, space='PSUM') as ps_small,
                tc.tile_pool(name='p1', bufs=1) as apool,
            ):
                x_sb = apool.tile([CIN, NPG], F32, tag='x_sb')
                nc.sync.dma_start(x_sb[:], xp_d.ap())
                xbf = apool.tile([CIN, NPG], BF16, tag='xbf')
                nc.vector.tensor_copy(xbf[:], x_sb[:])
                ident = apool.tile([128, 128], F32, tag='ident')
                nc.sync.dma_start(ident[:], ident_d.ap())
                woffT = apool.tile([CIN, 9, 18], BF16, tag='woffT')
                nc.sync.dma_start(
                    woffT[:], woffT_d.ap().rearrange('s c o -> c s o'))
                boff = apool.tile([18, 1], F32, tag='boff')
                nc.sync.dma_start(boff[:], boff_d.ap())
                hg1k = apool.tile([128, NJT, NK], F32, tag='hg1k')
                nc.sync.dma_start(hg1k[:], hg1k_d.ap())
                wg1k = apool.tile([128, NJT, NK], F32, tag='wg1k')
                nc.sync.dma_start(wg1k[:], wg1k_d.ap())
                nc.vector.tensor_copy(identb[:], ident[:])

                # ---------- offset conv: off [18, 4096] f32 ----------
                off_sb = apool.tile([18, HW], F32, tag='off_sb')
                chunks = [(1 + 7 * i, 7) for i in range(9)] + [(64, 1)]
                for r0, nr in chunks:
                    nfree = 66 * (nr - 1) + 64
                    ps = ps_small.tile([18, 512], F32, tag='ps', name='ps')
                    for s in range(9):
                        dh, dw = s // 3, s % 3
                        beg = r0 * 66 + 1 + (dh - 1) * 66 + (dw - 1)
                        rhs = bass.AP(
                            tensor=xbf[:].tensor,
                            offset=xbf[:].offset + beg,
                            ap=[list(xbf[:].ap[0]), [1, nfree]],
                        )
                        nc.tensor.matmul(
                            ps[:, :nfree], lhsT=woffT[:, s, :], rhs=rhs,
                            start=(s == 0), stop=(s == 8))
                    src_in = bass.AP(
                        tensor=ps[:].tensor, offset=ps[:].offset,
                        ap=[list(ps[:].ap[0]), [66, nr], [1, 64]])
                    nc.scalar.activation(
                        off_sb[:, 64 * (r0 - 1):64 * (r0 - 1 + nr)], src_in,
                        ACTF.Identity, bias=boff[:], scale=1.0)

                # ---------- transpose offsets to pixel-major ----------
                offT = apool.tile([128, NJT, 18], F32, tag='offT')
                for jt in range(NJT):
                    ps = ps_small.tile([128, 18], F32, tag='ps')
                    nc.tensor.transpose(
                        ps[:], off_sb[:, 128 * jt:128 * (jt + 1)],
                        ident[:18, :18])
                    nc.vector.tensor_copy(offT[:, jt, :], ps[:])

                # ---------- xT2 paired-row build ----------
                # For each padded row yy: transpose the 65-wide row pair
                # (yy, yy+1) to pixel-major into one PSUM tile; ACT copies
                # it out; one DMA writes the interleaved [r, 256] layout
                # (r = yy*65+xx). Emitted here so the PE work fills the gap
                # while DVE runs the index math below.
                xT2sb = apool.tile([65, 65, 2 * CIN], BF16, tag='xT2sb')
                for yy in range(65):
                    psAB = ps_small.tile([128, 2 * CIN], BF16, tag='psAB',
                                         name='ps')
                    nc.tensor.transpose(
                        psAB[:65, 0:CIN], xbf[:, 66 * yy:66 * yy + 65],
                        identb[:])
                    nc.tensor.transpose(
                        psAB[:65, CIN:2 * CIN],
                        xbf[:, 66 * yy + 66:66 * yy + 131], identb[:])
                    nc.scalar.activation(
                        xT2sb[:, yy, :], psAB[:65, :], ACTF.Copy)
                nc.sync.dma_start(
                    bass.AP(tensor=xT2_d, offset=0,
                            ap=[[2 * CIN, 65], [65 * 2 * CIN, 65],
                                [1, 2 * CIN]]),
                    xT2sb[:])
                zpad = apool.tile([1, 2 * CIN], BF16, tag='zpad')
                nc.vector.memset(zpad[:], 0)
                nc.sync.dma_start(
                    bass.AP(tensor=xT2_d, offset=(R2 - 1) * 2 * CIN,
                            ap=[[2 * CIN, 1], [1, 2 * CIN]]),
                    zpad[:])

                # ---------- index/weight arithmetic (pixel-major) ----------
                sh = [128, NJT, NK]

                def T(tag, dt=F32):
                    return apool.tile(sh, dt, tag=tag, name=tag)

                dyx = offT[:].rearrange('p jt (k two) -> p jt k two', two=2)
                dy = dyx[:, :, :, 0]
                dx = dyx[:, :, :, 1]

                ti = apool.tile(sh, I32, tag='ti')
                fdy, fdx = T('fdy'), T('fdx')
                tmp1, tmp2 = T('tmp1'), T('tmp2')
                # floor(dy)
                nc.vector.tensor_copy(ti[:], dy)
                nc.vector.tensor_copy(fdy[:], ti[:])
                nc.vector.tensor_tensor(tmp1[:], fdy[:], dy, ALU.is_gt)
                nc.vector.tensor_tensor(fdy[:], fdy[:], tmp1[:], ALU.subtract)
                # floor(dx)
                nc.vector.tensor_copy(ti[:], dx)
                nc.vector.tensor_copy(fdx[:], ti[:])
                nc.vector.tensor_tensor(tmp1[:], fdx[:], dx, ALU.is_gt)
                nc.vector.tensor_tensor(fdx[:], fdx[:], tmp1[:], ALU.subtract)

                ly, lx = T('ly'), T('lx')
                nc.vector.tensor_tensor(ly[:], dy, fdy[:], ALU.subtract)
                nc.vector.tensor_tensor(lx[:], dx, fdx[:], ALU.subtract)

                # ry = h + ky + fdy + 1 (base row of the gathered pair);
                # y0 = ry - 1, y1 = ry
                ry, rx = T('ry'), T('rx')
                nc.vector.tensor_tensor(ry[:], hg1k[:], fdy[:], ALU.add)
                nc.vector.tensor_tensor(rx[:], wg1k[:], fdx[:], ALU.add)

                y0, x0 = T('y0'), T('x0')
                nc.vector.tensor_scalar(y0[:], ry[:], 1.0, None, ALU.subtract)
                nc.vector.tensor_scalar(x0[:], rx[:], 1.0, None, ALU.subtract)

                vy0, vy1, vx0, vx1 = T('vy0'), T('vy1'), T('vx0'), T('vx1')
                nc.vector.tensor_scalar(tmp1[:], y0[:], 63.0, 0.0, ALU.min, ALU.max)
                nc.vector.tensor_tensor(vy0[:], y0[:], tmp1[:], ALU.is_equal)
                nc.vector.tensor_scalar(tmp1[:], ry[:], 63.0, 0.0, ALU.min, ALU.max)
                nc.vector.tensor_tensor(vy1[:], ry[:], tmp1[:], ALU.is_equal)
                nc.vector.tensor_scalar(tmp1[:], x0[:], 63.0, 0.0, ALU.min, ALU.max)
                nc.vector.tensor_tensor(vx0[:], x0[:], tmp1[:], ALU.is_equal)
                nc.vector.tensor_scalar(tmp1[:], rx[:], 63.0, 0.0, ALU.min, ALU.max)
                nc.vector.tensor_tensor(vx1[:], rx[:], tmp1[:], ALU.is_equal)

                # tap weights: wy0 = (1-ly)*vy0, wy1 = ly*vy1 (x analogous)
                wy0, wy1, wx0, wx1 = T('wy0'), T('wy1'), T('wx0'), T('wx1')
                nc.vector.tensor_tensor(tmp1[:], ly[:], vy0[:], ALU.mult)
                nc.vector.tensor_tensor(wy0[:], vy0[:], tmp1[:], ALU.subtract)
                nc.vector.tensor_tensor(wy1[:], ly[:], vy1[:], ALU.mult)
                nc.vector.tensor_tensor(tmp1[:], lx[:], vx0[:], ALU.mult)
                nc.vector.tensor_tensor(wx0[:], vx0[:], tmp1[:], ALU.subtract)
                nc.vector.tensor_tensor(wx1[:], lx[:], vx1[:], ALU.mult)

                # w4 (tap order matches the gathered elem layout:
                # [y0x0, y1x0, y0x1, y1x1])
                w4 = apool.tile([128, NJT, NK, 4], F32, tag='w4')
                nc.vector.tensor_tensor(w4[:, :, :, 0], wy0[:], wx0[:], ALU.mult)
                nc.vector.tensor_tensor(w4[:, :, :, 1], wy1[:], wx0[:], ALU.mult)
                nc.vector.tensor_tensor(w4[:, :, :, 2], wy0[:], wx1[:], ALU.mult)
                nc.vector.tensor_tensor(w4[:, :, :, 3], wy1[:], wx1[:], ALU.mult)
                nc.vector.tensor_copy(w4b[:], w4[:])

                # gather index = clip(ry,0,64)*65 + clip(rx,0,64)
                idxf = T('idxf')
                nc.vector.tensor_scalar(tmp1[:], ry[:], 64.0, 0.0, ALU.min, ALU.max)
                nc.vector.tensor_scalar(tmp2[:], rx[:], 64.0, 0.0, ALU.min, ALU.max)
                nc.vector.tensor_scalar(tmp1[:], tmp1[:], 65.0, None, ALU.mult)
                nc.vector.tensor_tensor(idxf[:], tmp1[:], tmp2[:], ALU.add)

                # ---------- per-k: un-transpose idx, wrap for the gather ----
                # Gather (k, half) runs on Q7 core pair q = (half*NK+k)%4,
                # which reads indices only from partitions [32q, 32q+32):
                # wrap directly into that group (two 16-partition xbar
                # transposes, split across Sync and Scalar HWDGE).
                idx_tr = apool.tile([NJT, NK, 128], I16, tag='idx_tr')
                for k in range(NK):
                    ps = ps_small.tile([NJT, 128], F32, tag='ps')
                    nc.tensor.transpose(ps[:], idxf[:, :, k], ident[:])
                    nc.vector.tensor_copy(idx_tr[:, k, :], ps[:])
                    nc.sync.dma_start(
                        idxnat_d.ap()[k, :].rearrange(
                            '(jt jp) -> jt jp', jp=128),
                        idx_tr[:, k, :])
                    for half in range(NHALF):
                        q = (half * NK + k) % 4
                        src = idxnat_d.ap()[k, half * JH:(half + 1) * JH]
                        nc.sync.dma_start_transpose(
                            idxwr[32 * q:32 * q + 16, k, half, :],
                            src.rearrange('(f p) -> f p', p=16))
                        nc.scalar.dma_start_transpose(
                            idxwr[32 * q + 16:32 * q + 32, k, half, :],
                            src.rearrange('(f p) -> f p', p=16))

            if phase != 1:
                _phase2(nc, tc, wdefT, bdef, idxwr, w4b, identb,
                        xT2_d, y_d, phase)
            if phase != 3:
                with tc.tile_pool(name='zz', bufs=1) as zp:
                    zt = zp.tile([128, HW], F32, tag='zt')
                    nc.vector.memset(zt[:], 0)
                    for oh in range(2):
                        nc.sync.dma_start(
                            y_d.ap()[128 * oh:128 * (oh + 1), :], zt[:])

    nc.finalize()
    _split_multiwaits(nc)
    return nc


def _phase2(nc, tc, wdefT, bdef, idxwr, w4b, identb, xT2_d, y_d, phase=3):
    # ======== phase 2: gather + weighting + conv ========
    with (
        tc.tile_pool(name='ps_tr', bufs=4, space='PSUM') as ps_tr,
        tc.tile_pool(name='ps_conv', bufs=2, space='PSUM') as ps_conv,
        tc.tile_pool(name='gath', bufs=3) as gpool,
        tc.tile_pool(name='prodp', bufs=2) as tpool,
        tc.tile_pool(name='val', bufs=2) as vpool,
        tc.tile_pool(name='outp', bufs=2) as opool,
    ):
        w4b_ap = w4b[:]
        for half in range(NHALF):
            j0 = half * JH
            valT = vpool.tile([128, NK, JH // 128, 128], BF16, tag='valT')
            for k in range(NK):
                # one descriptor per (pixel, k): elem = xT2 rows (r, r+1)
                # = 512 bf16 = the 2x2 bilinear window over all channels
                G = gpool.tile([128, JH // 128, 4 * CIN], BF16, tag='G',
                               name='G')
                in_ap = bass.AP(
                    tensor=xT2_d, offset=0,
                    ap=[[2 * CIN, R2 - 1], [1, 4 * CIN]],
                )
                nc.gpsimd.dma_gather(
                    out_ap=G[:],
                    in_ap=in_ap,
                    idxs_ap=idxwr[:, k, half, :],
                    num_idxs=JH,
                    num_idxs_reg=JH,
                    elem_size=4 * CIN,
                    elem_step=2 * CIN,
                    transpose=False,
                    single_packet=False,
                    queue_num=(half * NK + k) % 4,
                )
                if phase == 21:
                    continue
                # tap weighting: one DVE op per (k, half); the weight AP
                # broadcasts each w4 value across the 128-channel free dim
                prod = tpool.tile([128, JH // 128, 4, CIN], BF16, tag='prod',
                                  name='prod')
                wap = bass.AP(
                    tensor=w4b_ap.tensor,
                    offset=(w4b_ap.offset + (half * (JH // 128)) * (NK * 4)
                            + k * 4),
                    ap=[list(w4b_ap.ap[0]), [NK * 4, JH // 128], [1, 4],
                        [0, CIN]],
                )
                gv = G[:].rearrange('p j (t c) -> p j t c', t=4)
                nc.vector.tensor_tensor(prod[:], gv, wap, ALU.mult)
                # 4-tap sum + transpose to channel-major, fused on PE
                for jtl in range(JH // 128):
                    ps = ps_tr.tile([128, 128], F32, tag='pst', name='ps')
                    for t in range(4):
                        nc.tensor.matmul(
                            ps[:], lhsT=prod[:, jtl, t, :], rhs=identb[:],
                            start=(t == 0), stop=(t == 3))
                    nc.scalar.activation(valT[:, k, jtl, :], ps[:], ACTF.Copy)

            if phase != 3:
                continue
            valT_ap = valT[:]
            for jc in range(JH // 512):
                for oh in range(2):
                    psc = ps_conv.tile([128, 512], F32, tag='ps_conv')
                    for k in range(NK):
                        rhs = bass.AP(
                            tensor=valT_ap.tensor,
                            offset=valT_ap.offset + k * JH + jc * 512,
                            ap=[list(valT_ap.ap[0]), [1, 512]],
                        )
                        nc.tensor.matmul(
                            psc[:],
                            lhsT=wdefT[:, k, 128 * oh:128 * (oh + 1)],
                            rhs=rhs,
                            start=(k == 0), stop=(k == NK - 1))
                    yo = opool.tile([128, 512], F32, tag='yo')
                    nc.scalar.activation(
                        yo[:], psc[:], ACTF.Relu,
                        bias=bdef[:, oh:oh + 1], scale=1.0)
                    nc.scalar.dma_start(
                        y_d.ap()[128 * oh:128 * (oh + 1),
                                 j0 + 512 * jc:j0 + 512 * (jc + 1)],
                        yo[:])


def _host_prep(x, w_off, b_off, w_def, b_def):
    """Build per-core input maps."""
    x = np.asarray(x, np.float32)
    w_off = np.asarray(w_off, np.float32)
    b_off = np.asarray(b_off, np.float32)
    w_def = np.asarray(w_def, np.float32)
    b_def = np.asarray(b_def, np.float32)

    woffT = np.stack([w_off[:, :, s // 3, s % 3].T for s in range(9)])
    woffT = _to_bf16(np.ascontiguousarray(woffT, np.float32))  # [9, 128, 18]
    wdefT = np.stack([w_def[:, :, s // 3, s % 3].T for s in range(9)])
    wdefT = _to_bf16(np.ascontiguousarray(wdefT))             # [9, 128, 256]
    bdef2 = np.ascontiguousarray(b_def.reshape(2, 128).T)     # [128, 2]
    ident = np.eye(128, dtype=np.float32)

    jp = np.arange(128)[:, None, None]
    jt = np.arange(NJT)[None, :, None]
    kk = np.arange(NK)[None, None, :]
    j = jt * 128 + jp
    ky = np.array(KY, np.float32)[kk]
    kx = np.array(KX, np.float32)[kk]
    hg1k = (j // 64).astype(np.float32) + ky + 1.0
    wg1k = (j % 64).astype(np.float32) + kx + 1.0
    hg1k = np.ascontiguousarray(np.broadcast_to(hg1k, (128, NJT, NK)), np.float32)
    wg1k = np.ascontiguousarray(np.broadcast_to(wg1k, (128, NJT, NK)), np.float32)

    xp = np.pad(x, ((0, 0), (0, 0), (1, 1), (1, 1))).reshape(B, CIN, NPG)

    shared = {
        'woffT': woffT,
        'boff': np.ascontiguousarray(b_off.reshape(18, 1)),
        'wdefT': wdefT,
        'bdef': bdef2,
        'ident': ident,
        'hg1k': hg1k,
        'wg1k': wg1k,
    }
    in_maps = []
    for b in range(B):
        m = dict(shared)
        m['xp'] = np.ascontiguousarray(xp[b])
        in_maps.append(m)
    return in_maps


def _to_bf16(a):
    import ml_dtypes
    return a.astype(ml_dtypes.bfloat16)


LAST_RESULTS = None


def _ensure_trace_support():
    """Register the NTFF profile hook that the slim agent image lacks, and
    stub out the artifact upload. Only used when KBENCH_TRACE is set."""
    import contextlib
    import ctypes
    import types

    import concourse.bass_utils as bu
    bu.upload_artifacts = lambda tmpdir: tmpdir

    if 'antenv.axon_hooks' in sys.modules:
        return
    so_path = '/opt/axon/libaxon_pjrt.so'
    if not os.path.exists(so_path):
        return
    lib = ctypes.CDLL(so_path)
    if not hasattr(lib, 'axon_start_nrt_profile'):
        return
    lib.axon_start_nrt_profile.argtypes = [
        ctypes.POINTER(ctypes.c_int64), ctypes.c_size_t]
    lib.axon_start_nrt_profile.restype = ctypes.c_int64
    lib.axon_stop_nrt_profile.argtypes = [ctypes.c_char_p]
    lib.axon_stop_nrt_profile.restype = ctypes.c_int64

    @contextlib.contextmanager
    def _hook(output_dir, device_ids):
        import jax
        jax.devices()
        if device_ids:
            ids = (ctypes.c_int64 * len(device_ids))(*device_ids)
            rc = lib.axon_start_nrt_profile(ids, len(device_ids))
        else:
            rc = lib.axon_start_nrt_profile(None, 0)
        if rc != 0:
            raise RuntimeError(f'axon_start_nrt_profile rc={rc}')
        try:
            yield
        finally:
            n = lib.axon_stop_nrt_profile(str(output_dir).encode())
            print(f'profile: {n} file(s) written to {output_dir}',
                  file=sys.stderr)

    mod = types.ModuleType('antenv.axon_hooks')
    mod.get_axon_ntff_profile_hook = lambda: _hook
    mod.set_axon_ntff_profile_hook = lambda h: None
    sys.modules['antenv.axon_hooks'] = mod


def kernel(x, w_off, b_off, w_def, b_def):
    global LAST_RESULTS
    if 'nc' not in _CACHE:
        _CACHE['nc'] = _build_program(
            phase=int(os.environ.get('KBENCH_PHASE', '3')))
    nc = _CACHE['nc']
    in_maps = _host_prep(x, w_off, b_off, w_def, b_def)
    trace = bool(os.environ.get('KBENCH_TRACE'))
    if trace:
        _ensure_trace_support()
    res = run_bass_kernel_spmd(
        nc, in_maps, core_ids=list(range(B)),
        trace=trace,
    )
    LAST_RESULTS = res
    out = np.stack([res.results[b]['y'].reshape(COUT, H, W) for b in range(B)])
    return out.astype(np.float32)
